# revision 6
# baseline (speedup 1.0000x reference)
"""Trainium2 Bass kernel for nn_BinaryTreeLogicNet (v2: custom-DVE level 0).

Math (x:[B,256], W_leaf:[256,256], weights:[255,2], biases:[255],
w_out:[1,1], b_out:[1]):

    leaf = sigmoid(x @ W_leaf.T - 2)                       # (B, 256)
    8-level pairwise tree reduce with generalized-gcd nodes # (B, 1)
    out  = sigmoid(root * w_out + b_out)

All tree values are positive, so each node is
    node = A*(l+r) + C*max(l,r),  A = lam*k, C = k*(1-2*lam)
(k = consumer weight folded in).  Per-core structure (B/8 rows):

  1. Matmul with W stationary and xT streamed ("orientation-2"): psum is
     [leaf-node partitions, batch free].  Leaf pairs are split across two
     psum tiles (left children -> uA, right -> uB) in bit-reversed node
     order, so level-0 is a per-partition op.
  2. ScalarE sigmoid psum->SBUF fp16.
  3. Level 0 runs as ONE custom DVE instruction (TREELEAF:
     out = (C0*in0 + in1) + C1*max(C0*in0, in1), per-partition C0/C1),
     folding the per-leaf weights and the node constants; that is ~3x
     cheaper than the stock wint-mult + 4-op level.
  4. A 4x-mode tensor_scalar rescales v0 to the sigma chain the stock
     levels expect; a DMA xbar transpose moves v0 [128, F] to batch-major
     [128, F/128, 128].
  5. Levels 1-7 run batch-major exactly like the v1 kernel (4 tensor_tensor
     per level on bit-reversed halves; root with explicit A', C').
  6. Final sigmoid(root + b_out) on ScalarE, DMA out.

Sharding: pure data parallel over batch across 8 cores; x transposed and
cast to bf16 on the host so the contraction dim is on partitions.
"""

import numpy as np

import concourse.bass as bass
import concourse.bacc as bacc
import concourse.mybir as mybir
import concourse.tile as tile
from concourse.bass_utils import run_bass_kernel_spmd

# ---- custom DVE op (registered into the concourse catalog at import) ------
import concourse.dve_ops as dve_ops
from concourse.dve_spec import Spec, Src0, Src1, C0, C1, maxx, lower, _has_src1
from concourse.dve_uop import DveOpSpec


def _register_treeleaf():
    name = "TREELEAF_ANT"
    if name in dve_ops._SUB_OPCODE_FOR_NAME:
        for op in dve_ops.OPS:
            if op.name == name:
                return op
        raise RuntimeError(name)
    t = Src0 * C0
    spec = Spec(
        body=(t + Src1) + C1 * maxx(t, Src1),
        reference=lambda in0, in1, s0, s1, imm2: (
            in0.astype(np.float32) * s0 + in1.astype(np.float32)
        )
        + s1 * np.maximum(in0.astype(np.float32) * s0, in1.astype(np.float32)),
    )
    row = dve_ops._CUSTOM_DVE_ROW_BASE + len(dve_ops.OPS)
    assert row < 0x20
    shas = {}
    for ver in ("v3", "v4"):
        s = DveOpSpec(
            name=name, opcode=row, uops=lower(spec, ver=ver), rd1_en=_has_src1(spec)
        )
        shas[ver] = s.sha(ver)
    op = dve_ops.DveOp(name, spec, subdim=False, uops_sha=shas)
    dve_ops.OPS.append(op)
    dve_ops._SUB_OPCODE_FOR_NAME[name] = row
    dve_ops.CUSTOM_DVE_SPECS[name] = spec
    return op


TREELEAF = _register_treeleaf()

# ---- problem geometry (hardcoded per contract) ----
B, L = 65536, 256
N_CORES = 8
BS = B // N_CORES            # 8192 rows per core
TILES = BS // 128            # 64 tiles of 128 rows
SC = 2048                    # super-chunk batch columns
NSC = BS // SC               # 4 super-chunks
PC = 1024                    # psum chunk (2 banks) for matmul/sigmoid
MMF = 512                    # matmul free size per instruction
RHO = 128.0                  # pow2 rescale anchoring the stock sigma chain

EPS = 1e-6
SHARPNESS = 1.0
BIAS_SHIFT = -2.0

MM_DT = mybir.dt.bfloat16
TREE_DT = mybir.dt.float16
CST_DT = mybir.dt.float16


def _sigmoid(z):
    return 1.0 / (1.0 + np.exp(-z))


def _levels():
    out, off, m = [], 0, 128
    while m >= 1:
        out.append((off, m))
        off += m
        m //= 2
    return out


def _bitrev(n):
    bits = n.bit_length() - 1
    out = np.zeros(n, np.int64)
    for j in range(n):
        r, x = 0, j
        for _ in range(bits):
            r = (r << 1) | (x & 1)
            x >>= 1
        out[j] = r
    return out


def prep_consts(weights, biases, w_out):
    """Host-folded constants (float64), all in bit-reversed position order.

    Returns dict with:
      a0[128], ch0[128]  L0 TREELEAF consts
      fix0[128]          v0 rescale onto the stock sigma-chain target
      chat_cat[126]      Chat for levels 1..6, concatenated
      a7, c7             root level explicit consts (on RHO scale)
    """
    w = weights.astype(np.float64)
    b = biases.astype(np.float64)
    lv = _levels()
    A_lv, C_lv, WL, WR = [], [], [], []
    for li, (off, m) in enumerate(lv):
        lam = _sigmoid(b[off : off + m])
        if li + 1 < len(lv):
            noff, nm = lv[li + 1]
            k = np.empty(m, np.float64)
            k[0::2] = w[noff : noff + nm, 0]
            k[1::2] = w[noff : noff + nm, 1]
        else:
            k = np.full(m, float(w_out[0, 0]), np.float64)
        A_lv.append(lam * k)
        C_lv.append(k * (1.0 - 2.0 * lam))
        WL.append(w[off : off + m, 0].copy())
        WR.append(w[off : off + m, 1].copy())

    # stock sigma-chain targets: sig[li][j] = stored scale of level-li node j
    sig = [None] * 7
    sig[6] = np.full(2, RHO)
    for li in range(5, -1, -1):
        j = np.arange(128 >> li)
        sig[li] = sig[li + 1][j >> 1] * A_lv[li + 1][j >> 1]

    # custom L0: v0 = (a0*uA + uB) + ch0*max(.)  => exact0 = sigma0_a * v0
    a0 = WL[0] / WR[0]
    ch0 = C_lv[0] / A_lv[0]
    sigma0_a = A_lv[0] * WR[0]
    # corrected v0' = v0 * fix0 is stored at the stock target scale sig[0]
    fix0 = sigma0_a / sig[0]

    chat_parts = [(C_lv[li] / A_lv[li])[_bitrev(128 >> li)] for li in range(1, 7)]
    a7 = float(A_lv[7][0] / RHO)
    c7 = float(C_lv[7][0] / RHO)
    return {
        "a0": a0[_bitrev(128)],
        "ch0": ch0[_bitrev(128)],
        "fix0": fix0[_bitrev(128)],
        "chat_cat": np.concatenate(chat_parts),
        "a7": a7,
        "c7": c7,
    }


def host_emulate(x, W_leaf, weights, biases, w_out, b_out, dtype=np.float16):
    """Numpy emulation of the kernel math/layout for validation."""
    cst = prep_consts(weights, biases, w_out)
    br128 = _bitrev(128)
    lA = 2 * br128
    lB = lA + 1
    xf = x.astype(np.float32)
    zA = xf @ W_leaf[lA].T.astype(np.float32) + np.float32(BIAS_SHIFT)
    zB = xf @ W_leaf[lB].T.astype(np.float32) + np.float32(BIAS_SHIFT)
    uA = _sigmoid(zA).astype(dtype).astype(np.float32)
    uB = _sigmoid(zB).astype(dtype).astype(np.float32)
    a0 = cst["a0"].astype(np.float32)
    ch0 = cst["ch0"].astype(np.float32)
    t = uA * a0  # fp32 internally in the custom op
    v0 = ((t + uB) + ch0 * np.maximum(t, uB)).astype(dtype)
    v0f = (v0.astype(np.float32) * cst["fix0"].astype(np.float32)).astype(dtype)
    cur = v0f
    off = 0
    for li in range(1, 7):
        m = 128 >> li
        l_, r_ = cur[:, 0:m], cur[:, m : 2 * m]
        s = (l_.astype(np.float32) + r_.astype(np.float32)).astype(dtype)
        mx = np.maximum(l_, r_)
        Ch = cst["chat_cat"][off : off + m].astype(dtype)
        cur = (
            s.astype(np.float32)
            + (mx.astype(np.float32) * Ch.astype(np.float32))
            .astype(dtype)
            .astype(np.float32)
        ).astype(dtype)
        off += m
    l_, r_ = cur[:, 0:1].astype(np.float32), cur[:, 1:2].astype(np.float32)
    s = (l_ + r_).astype(dtype).astype(np.float32)
    mx = np.maximum(l_, r_)
    root = (
        s * np.float32(cst["a7"]) + (mx * np.float32(cst["c7"])).astype(dtype)
    ).astype(np.float32)
    return _sigmoid(root + np.float32(b_out[0]))


def build_nc(b_out_val, a7, c7):
    nc = bacc.Bacc("TRN2", target_bir_lowering=False, debug=False)

    xt = nc.dram_tensor("xt", [2, 128, BS], MM_DT, kind="ExternalInput")
    # 4 stationaries [k 128, {WA0,WA1,WB0,WB1}, j 128]
    wst = nc.dram_tensor("wst", [128, 4, 128], MM_DT, kind="ExternalInput")
    # per-partition consts (fp32): a0, ch0, fix0
    ppc = nc.dram_tensor("ppc", [128, 3], mybir.dt.float32, kind="ExternalInput")
    # batch-major const row: chat_cat(126) | pad, replicated on partitions
    cst = nc.dram_tensor("cst", [128, 128], CST_DT, kind="ExternalInput")
    outp = nc.dram_tensor("out", [128, TILES], mybir.dt.float32, kind="ExternalOutput")

    SCT = SC // 128  # tiles per super-chunk (16)

    with tile.TileContext(nc) as tc:
        with (
            tc.tile_pool(name="const", bufs=1) as constp,
            tc.tile_pool(name="xload", bufs=3) as xp,
            tc.tile_pool(name="u", bufs=2) as up,
            tc.tile_pool(name="v", bufs=2) as vp,
            tc.tile_pool(name="bm", bufs=1) as bmp,
            tc.tile_pool(name="ps", bufs=2, space="PSUM") as psp,
        ):
            wsb = constp.tile([128, 4, 128], MM_DT)
            nc.sync.dma_start(out=wsb[:, :, :], in_=wst.ap())
            ppc_sb = constp.tile([128, 3], mybir.dt.float32)
            nc.sync.dma_start(out=ppc_sb[:, :], in_=ppc.ap())
            cst_sb = constp.tile([128, 128], CST_DT)
            nc.sync.dma_start(out=cst_sb[:, :], in_=cst.ap())
            bias_shift = constp.tile([128, 1], mybir.dt.float32)
            nc.vector.memset(bias_shift[:, :], float(BIAS_SHIFT))
            bias_out = constp.tile([128, 1], mybir.dt.float32)
            nc.vector.memset(bias_out[:, :], float(b_out_val))

            # batch-major storage for the whole core
            v0T = bmp.tile([128, TILES, 128], TREE_DT)
            roots = bmp.tile([128, TILES], TREE_DT)
            scr = bmp.tile([128, TILES, 192], TREE_DT)

            def bconst(lo, n, T, toff):
                return (
                    cst_sb[:, lo : lo + n]
                    .rearrange("p (o w) -> p o w", o=1)
                    .broadcast_to([128, T, n])
                )

            def stock_group(tsl, T):
                """Levels 1..6 + root on v0T[:, tsl, :]."""
                cur = v0T[:, tsl, :]
                off = 0
                for li2 in range(6):
                    m = 64 >> li2
                    le = cur[:, :, 0:m]
                    ro = cur[:, :, m : 2 * m]
                    s = scr[:, tsl, 0:m]
                    mx = scr[:, tsl, 64 : 64 + m]
                    q2 = scr[:, tsl, 128 : 128 + m]
                    nc.vector.tensor_tensor(
                        out=s, in0=le, in1=ro, op=mybir.AluOpType.add
                    )
                    nc.vector.tensor_tensor(
                        out=mx, in0=le, in1=ro, op=mybir.AluOpType.max
                    )
                    nc.vector.tensor_tensor(
                        out=q2,
                        in0=mx,
                        in1=bconst(off, m, T, tsl),
                        op=mybir.AluOpType.mult,
                    )
                    nc.vector.tensor_tensor(
                        out=cur[:, :, 0:m], in0=s, in1=q2, op=mybir.AluOpType.add
                    )
                    off += m
                s = scr[:, tsl, 0:1]
                mx = scr[:, tsl, 64:65]
                q2 = scr[:, tsl, 128:129]
                nc.vector.tensor_tensor(
                    out=s, in0=cur[:, :, 0:1], in1=cur[:, :, 1:2],
                    op=mybir.AluOpType.add,
                )
                nc.vector.tensor_tensor(
                    out=mx, in0=cur[:, :, 0:1], in1=cur[:, :, 1:2],
                    op=mybir.AluOpType.max,
                )
                nc.vector.tensor_scalar_mul(out=q2, in0=mx, scalar1=float(c7))
                rsl = roots[:, tsl].rearrange("p (t o) -> p t o", o=1)
                nc.vector.scalar_tensor_tensor(
                    out=rsl,
                    in0=s,
                    scalar=float(a7),
                    in1=q2,
                    op0=mybir.AluOpType.mult,
                    op1=mybir.AluOpType.add,
                )

            # chunk/group schedule: small first chunk primes the pipeline;
            # stock groups fire once their tiles' transposes are queued.
            CHUNKS = [512, 1536, 2048, 2048, 2048]
            GROUP_AT = {2: (0, 16), 3: (16, 16), 5: (32, 32)}  # after chunk i
            xoff = 0
            for ci, CW in enumerate(CHUNKS):
                xa = xp.tile([128, CW], MM_DT, tag=f"xa{CW}")
                xb = xp.tile([128, CW], MM_DT, tag=f"xb{CW}")
                nc.sync.dma_start(out=xa[:, :], in_=xt.ap()[0, :, xoff : xoff + CW])
                nc.sync.dma_start(out=xb[:, :], in_=xt.ap()[1, :, xoff : xoff + CW])

                uA = up.tile([128, CW], TREE_DT, tag=f"uA{CW}")
                uB = up.tile([128, CW], TREE_DT, tag=f"uB{CW}")
                done = 0
                while done < CW:
                    pw = min(PC, CW - done)
                    psA = psp.tile([128, pw], mybir.dt.float32, tag="psA")
                    psB = psp.tile([128, pw], mybir.dt.float32, tag="psB")
                    for half, ps in ((0, psA), (1, psB)):
                        for ki in range(2):
                            xsrc = xa if ki == 0 else xb
                            st = wsb[:, 2 * half + ki, :]
                            for f in range(0, pw, MMF):
                                fw = min(MMF, pw - f)
                                nc.tensor.matmul(
                                    ps[:, f : f + fw],
                                    st,
                                    xsrc[:, done + f : done + f + fw],
                                    start=(ki == 0),
                                    stop=(ki == 1),
                                )
                    nc.scalar.activation(
                        out=uA[:, done : done + pw],
                        in_=psA[:, :],
                        func=mybir.ActivationFunctionType.Sigmoid,
                        bias=bias_shift[:, :],
                        scale=float(SHARPNESS),
                    )
                    nc.scalar.activation(
                        out=uB[:, done : done + pw],
                        in_=psB[:, :],
                        func=mybir.ActivationFunctionType.Sigmoid,
                        bias=bias_shift[:, :],
                        scale=float(SHARPNESS),
                    )
                    done += pw

                # L0 custom: v0 = (a0*uA + uB) + ch0*max(a0*uA, uB)
                v0 = vp.tile([128, CW], TREE_DT, tag=f"v0{CW}")
                nc.vector._custom_dve(
                    TREELEAF,
                    out=v0[:, :],
                    in0=uA[:, :],
                    in1=uB[:, :],
                    s0=ppc_sb[:, 0:1],
                    s1=ppc_sb[:, 1:2],
                )
                # rescale onto the stock sigma chain (tensor_scalar, 4x)
                v0f = vp.tile([128, CW], TREE_DT, tag=f"v0f{CW}")
                nc.vector.tensor_scalar(
                    out=v0f[:, :],
                    in0=v0[:, :],
                    scalar1=ppc_sb[:, 2:3],
                    scalar2=None,
                    op0=mybir.AluOpType.mult,
                )
                # transpose [128, CW] -> batch-major [128, CW/128, 128]
                toff = xoff // 128
                nc.sync.dma_start_transpose(
                    v0T[:, toff : toff + CW // 128, :], v0f[:, :]
                )
                xoff += CW
                if ci + 1 in GROUP_AT:
                    t0, tn = GROUP_AT[ci + 1]
                    stock_group(slice(t0, t0 + tn), tn)

            final = constp.tile([128, TILES], mybir.dt.float32)
            nc.scalar.activation(
                out=final[:, :],
                in_=roots[:, :],
                func=mybir.ActivationFunctionType.Sigmoid,
                bias=bias_out[:, :],
                scale=1.0,
            )
            nc.sync.dma_start(out=outp.ap(), in_=final[:, :])

    nc.compile()
    return nc


def make_in_maps(x, W_leaf, weights, biases, w_out):
    import ml_dtypes

    np_mm = ml_dtypes.bfloat16
    cst = prep_consts(weights, biases, w_out)
    br128 = _bitrev(128)
    lA = 2 * br128
    lB = lA + 1

    Wf = W_leaf.astype(np.float32)
    WA = Wf[lA]  # [128 j, 256 k]
    WB = Wf[lB]
    wst = np.empty((128, 4, 128), np.float32)
    wst[:, 0, :] = WA[:, 0:128].T
    wst[:, 1, :] = WA[:, 128:256].T
    wst[:, 2, :] = WB[:, 0:128].T
    wst[:, 3, :] = WB[:, 128:256].T
    wst = np.ascontiguousarray(wst.astype(np_mm))

    ppc = np.zeros((128, 3), np.float32)
    ppc[:, 0] = cst["a0"]
    ppc[:, 1] = cst["ch0"]
    ppc[:, 2] = cst["fix0"]

    cst_row = np.zeros(128, np.float16)
    cst_row[0:126] = cst["chat_cat"].astype(np.float16)
    cst_np = np.ascontiguousarray(np.broadcast_to(cst_row, (128, 128)))

    xT = np.ascontiguousarray(x.T.astype(np_mm))  # [256, B]
    in_maps = []
    for c in range(N_CORES):
        sh = np.ascontiguousarray(xT[:, c * BS : (c + 1) * BS].reshape(2, 128, BS))
        in_maps.append({"xt": sh, "wst": wst, "ppc": ppc, "cst": cst_np})
    return in_maps, cst["a7"], cst["c7"]


def gather_out(results):
    full = np.empty((B, 1), np.float32)
    for c in range(N_CORES):
        r = np.asarray(results[c]["out"])  # [128, TILES]
        full[c * BS : (c + 1) * BS, 0] = r.T.reshape(BS)
    return full


def kernel(x, W_leaf, weights, biases, w_out, b_out, _run_kwargs=None):
    x = np.asarray(x, dtype=np.float32)
    W_leaf = np.asarray(W_leaf, dtype=np.float32)
    weights = np.asarray(weights, dtype=np.float32)
    biases = np.asarray(biases, dtype=np.float32)
    w_out = np.asarray(w_out, dtype=np.float32)
    b_out = np.asarray(b_out, dtype=np.float32)
    in_maps, a7, c7 = make_in_maps(x, W_leaf, weights, biases, w_out)
    nc = build_nc(float(b_out[0]), a7, c7)
    kw = dict(_run_kwargs or {})
    res = run_bass_kernel_spmd(nc, in_maps, core_ids=list(range(N_CORES)), **kw)
    out = gather_out(res.results)
    if _run_kwargs is not None:
        kernel.last_results = res
    return out


# revision 8
# speedup vs baseline: 1.0226x; 1.0226x over previous
"""Trainium2 Bass kernel for nn_BinaryTreeLogicNet (v2: custom-DVE level 0).

Math (x:[B,256], W_leaf:[256,256], weights:[255,2], biases:[255],
w_out:[1,1], b_out:[1]):

    leaf = sigmoid(x @ W_leaf.T - 2)                       # (B, 256)
    8-level pairwise tree reduce with generalized-gcd nodes # (B, 1)
    out  = sigmoid(root * w_out + b_out)

All tree values are positive, so each node is
    node = A*(l+r) + C*max(l,r),  A = lam*k, C = k*(1-2*lam)
(k = consumer weight folded in).  Per-core structure (B/8 rows):

  1. Matmul with W stationary and xT streamed ("orientation-2"): psum is
     [leaf-node partitions, batch free].  Leaf pairs are split across two
     psum tiles (left children -> uA, right -> uB) in bit-reversed node
     order, so level-0 is a per-partition op.
  2. ScalarE sigmoid psum->SBUF fp16.
  3. Level 0 runs as ONE custom DVE instruction (TREELEAF:
     out = (C0*in0 + in1) + C1*max(C0*in0, in1), per-partition C0/C1),
     folding the per-leaf weights and the node constants; that is ~3x
     cheaper than the stock wint-mult + 4-op level.
  4. A 4x-mode tensor_scalar rescales v0 to the sigma chain the stock
     levels expect; a DMA xbar transpose moves v0 [128, F] to batch-major
     [128, F/128, 128].
  5. Levels 1-7 run batch-major exactly like the v1 kernel (4 tensor_tensor
     per level on bit-reversed halves; root with explicit A', C').
  6. Final sigmoid(root + b_out) on ScalarE, DMA out.

Sharding: pure data parallel over batch across 8 cores; x transposed and
cast to bf16 on the host so the contraction dim is on partitions.
"""

import numpy as np

import concourse.bass as bass
import concourse.bacc as bacc
import concourse.mybir as mybir
import concourse.tile as tile
from concourse.bass_utils import run_bass_kernel_spmd

# ---- custom DVE op (registered into the concourse catalog at import) ------
import concourse.dve_ops as dve_ops
from concourse.dve_spec import Spec, Src0, Src1, C0, C1, maxx, lower, _has_src1
from concourse.dve_uop import DveOpSpec


def _register_treeleaf():
    name = "TREELEAF_ANT"
    if name in dve_ops._SUB_OPCODE_FOR_NAME:
        for op in dve_ops.OPS:
            if op.name == name:
                return op
        raise RuntimeError(name)
    t = Src0 * C0
    spec = Spec(
        body=(t + Src1) + C1 * maxx(t, Src1),
        reference=lambda in0, in1, s0, s1, imm2: (
            in0.astype(np.float32) * s0 + in1.astype(np.float32)
        )
        + s1 * np.maximum(in0.astype(np.float32) * s0, in1.astype(np.float32)),
    )
    row = dve_ops._CUSTOM_DVE_ROW_BASE + len(dve_ops.OPS)
    assert row < 0x20
    shas = {}
    for ver in ("v3", "v4"):
        s = DveOpSpec(
            name=name, opcode=row, uops=lower(spec, ver=ver), rd1_en=_has_src1(spec)
        )
        shas[ver] = s.sha(ver)
    op = dve_ops.DveOp(name, spec, subdim=False, uops_sha=shas)
    dve_ops.OPS.append(op)
    dve_ops._SUB_OPCODE_FOR_NAME[name] = row
    dve_ops.CUSTOM_DVE_SPECS[name] = spec
    return op


TREELEAF = _register_treeleaf()

# ---- problem geometry (hardcoded per contract) ----
B, L = 65536, 256
N_CORES = 8
BS = B // N_CORES            # 8192 rows per core
TILES = BS // 128            # 64 tiles of 128 rows
SC = 2048                    # super-chunk batch columns
NSC = BS // SC               # 4 super-chunks
PC = 1024                    # psum chunk (2 banks) for matmul/sigmoid
MMF = 512                    # matmul free size per instruction
RHO = 128.0                  # pow2 rescale anchoring the stock sigma chain

EPS = 1e-6
SHARPNESS = 1.0
BIAS_SHIFT = -2.0

MM_DT = mybir.dt.bfloat16
TREE_DT = mybir.dt.float16
CST_DT = mybir.dt.float16


def _sigmoid(z):
    return 1.0 / (1.0 + np.exp(-z))


def _levels():
    out, off, m = [], 0, 128
    while m >= 1:
        out.append((off, m))
        off += m
        m //= 2
    return out


def _bitrev(n):
    bits = n.bit_length() - 1
    out = np.zeros(n, np.int64)
    for j in range(n):
        r, x = 0, j
        for _ in range(bits):
            r = (r << 1) | (x & 1)
            x >>= 1
        out[j] = r
    return out


def prep_consts(weights, biases, w_out):
    """Host-folded constants (float64), all in bit-reversed position order.

    Returns dict with:
      a0[128], ch0[128]  L0 TREELEAF consts
      fix0[128]          v0 rescale onto the stock sigma-chain target
      chat_cat[126]      Chat for levels 1..6, concatenated
      a7, c7             root level explicit consts (on RHO scale)
    """
    w = weights.astype(np.float64)
    b = biases.astype(np.float64)
    lv = _levels()
    A_lv, C_lv, WL, WR = [], [], [], []
    for li, (off, m) in enumerate(lv):
        lam = _sigmoid(b[off : off + m])
        if li + 1 < len(lv):
            noff, nm = lv[li + 1]
            k = np.empty(m, np.float64)
            k[0::2] = w[noff : noff + nm, 0]
            k[1::2] = w[noff : noff + nm, 1]
        else:
            k = np.full(m, float(w_out[0, 0]), np.float64)
        A_lv.append(lam * k)
        C_lv.append(k * (1.0 - 2.0 * lam))
        WL.append(w[off : off + m, 0].copy())
        WR.append(w[off : off + m, 1].copy())

    # stock sigma-chain targets: sig[li][j] = stored scale of level-li node j
    sig = [None] * 7
    sig[6] = np.full(2, RHO)
    for li in range(5, -1, -1):
        j = np.arange(128 >> li)
        sig[li] = sig[li + 1][j >> 1] * A_lv[li + 1][j >> 1]

    # custom L0: v0 = (a0*uA + uB) + ch0*max(.)  => exact0 = sigma0_a * v0
    a0 = WL[0] / WR[0]
    ch0 = C_lv[0] / A_lv[0]
    sigma0_a = A_lv[0] * WR[0]
    # corrected v0' = v0 * fix0 is stored at the stock target scale sig[0]
    fix0 = sigma0_a / sig[0]

    chat_parts = [(C_lv[li] / A_lv[li])[_bitrev(128 >> li)] for li in range(1, 7)]
    a7 = float(A_lv[7][0] / RHO)
    c7 = float(C_lv[7][0] / RHO)
    return {
        "a0": a0[_bitrev(128)],
        "ch0": ch0[_bitrev(128)],
        "fix0": fix0[_bitrev(128)],
        "chat_cat": np.concatenate(chat_parts),
        "a7": a7,
        "c7": c7,
    }


def host_emulate(x, W_leaf, weights, biases, w_out, b_out, dtype=np.float16):
    """Numpy emulation of the kernel math/layout for validation."""
    cst = prep_consts(weights, biases, w_out)
    br128 = _bitrev(128)
    lA = 2 * br128
    lB = lA + 1
    xf = x.astype(np.float32)
    zA = xf @ W_leaf[lA].T.astype(np.float32) + np.float32(BIAS_SHIFT)
    zB = xf @ W_leaf[lB].T.astype(np.float32) + np.float32(BIAS_SHIFT)
    uA = _sigmoid(zA).astype(dtype).astype(np.float32)
    uB = _sigmoid(zB).astype(dtype).astype(np.float32)
    a0 = cst["a0"].astype(np.float32)
    ch0 = cst["ch0"].astype(np.float32)
    t = uA * a0  # fp32 internally in the custom op
    v0 = ((t + uB) + ch0 * np.maximum(t, uB)).astype(dtype)
    v0f = (v0.astype(np.float32) * cst["fix0"].astype(np.float32)).astype(dtype)
    cur = v0f
    off = 0
    for li in range(1, 7):
        m = 128 >> li
        l_, r_ = cur[:, 0:m], cur[:, m : 2 * m]
        s = (l_.astype(np.float32) + r_.astype(np.float32)).astype(dtype)
        mx = np.maximum(l_, r_)
        Ch = cst["chat_cat"][off : off + m].astype(dtype)
        cur = (
            s.astype(np.float32)
            + (mx.astype(np.float32) * Ch.astype(np.float32))
            .astype(dtype)
            .astype(np.float32)
        ).astype(dtype)
        off += m
    l_, r_ = cur[:, 0:1].astype(np.float32), cur[:, 1:2].astype(np.float32)
    s = (l_ + r_).astype(dtype).astype(np.float32)
    mx = np.maximum(l_, r_)
    root = (
        s * np.float32(cst["a7"]) + (mx * np.float32(cst["c7"])).astype(dtype)
    ).astype(np.float32)
    return _sigmoid(root + np.float32(b_out[0]))


def build_nc(b_out_val, a7, c7):
    nc = bacc.Bacc("TRN2", target_bir_lowering=False, debug=False)

    xt = nc.dram_tensor("xt", [2, 128, BS], MM_DT, kind="ExternalInput")
    # 4 stationaries [k 128, {WA0,WA1,WB0,WB1}, j 128]
    wst = nc.dram_tensor("wst", [128, 4, 128], MM_DT, kind="ExternalInput")
    # per-partition consts (fp32): a0, ch0, fix0
    ppc = nc.dram_tensor("ppc", [128, 3], mybir.dt.float32, kind="ExternalInput")
    # batch-major const row: chat_cat(126) | pad, replicated on partitions
    cst = nc.dram_tensor("cst", [128, 128], CST_DT, kind="ExternalInput")
    outp = nc.dram_tensor("out", [128, TILES], mybir.dt.float32, kind="ExternalOutput")

    SCT = SC // 128  # tiles per super-chunk (16)

    with tile.TileContext(nc) as tc:
        with (
            tc.tile_pool(name="const", bufs=1) as constp,
            tc.tile_pool(name="xload", bufs=3) as xp,
            tc.tile_pool(name="u", bufs=2) as up,
            tc.tile_pool(name="v", bufs=2) as vp,
            tc.tile_pool(name="bm", bufs=1) as bmp,
            tc.tile_pool(name="ps", bufs=1, space="PSUM") as psp,
        ):
            wsb = constp.tile([128, 4, 128], MM_DT)
            nc.sync.dma_start(out=wsb[:, :, :], in_=wst.ap())
            ppc_sb = constp.tile([128, 3], mybir.dt.float32)
            nc.sync.dma_start(out=ppc_sb[:, :], in_=ppc.ap())
            cst_sb = constp.tile([128, 128], CST_DT)
            nc.sync.dma_start(out=cst_sb[:, :], in_=cst.ap())
            bias_shift = constp.tile([128, 1], mybir.dt.float32)
            nc.vector.memset(bias_shift[:, :], float(BIAS_SHIFT))
            bias_out = constp.tile([128, 1], mybir.dt.float32)
            nc.vector.memset(bias_out[:, :], float(b_out_val))

            # batch-major storage for the whole core
            v0T = bmp.tile([128, TILES, 128], TREE_DT)
            roots = bmp.tile([128, TILES], TREE_DT)
            scr = bmp.tile([128, TILES, 192], TREE_DT)

            def bconst(lo, n, T, toff):
                return (
                    cst_sb[:, lo : lo + n]
                    .rearrange("p (o w) -> p o w", o=1)
                    .broadcast_to([128, T, n])
                )

            def stock_group(tsl, T):
                """Levels 1..6 + root on v0T[:, tsl, :]."""
                cur = v0T[:, tsl, :]
                off = 0
                for li2 in range(6):
                    m = 64 >> li2
                    le = cur[:, :, 0:m]
                    ro = cur[:, :, m : 2 * m]
                    s = scr[:, tsl, 0:m]
                    mx = scr[:, tsl, 64 : 64 + m]
                    q2 = scr[:, tsl, 128 : 128 + m]
                    nc.vector.tensor_tensor(
                        out=s, in0=le, in1=ro, op=mybir.AluOpType.add
                    )
                    nc.vector.tensor_tensor(
                        out=mx, in0=le, in1=ro, op=mybir.AluOpType.max
                    )
                    nc.vector.tensor_tensor(
                        out=q2,
                        in0=mx,
                        in1=bconst(off, m, T, tsl),
                        op=mybir.AluOpType.mult,
                    )
                    nc.vector.tensor_tensor(
                        out=cur[:, :, 0:m], in0=s, in1=q2, op=mybir.AluOpType.add
                    )
                    off += m
                s = scr[:, tsl, 0:1]
                mx = scr[:, tsl, 64:65]
                q2 = scr[:, tsl, 128:129]
                nc.vector.tensor_tensor(
                    out=s, in0=cur[:, :, 0:1], in1=cur[:, :, 1:2],
                    op=mybir.AluOpType.add,
                )
                nc.vector.tensor_tensor(
                    out=mx, in0=cur[:, :, 0:1], in1=cur[:, :, 1:2],
                    op=mybir.AluOpType.max,
                )
                nc.vector.tensor_scalar_mul(out=q2, in0=mx, scalar1=float(c7))
                rsl = roots[:, tsl].rearrange("p (t o) -> p t o", o=1)
                nc.vector.scalar_tensor_tensor(
                    out=rsl,
                    in0=s,
                    scalar=float(a7),
                    in1=q2,
                    op0=mybir.AluOpType.mult,
                    op1=mybir.AluOpType.add,
                )

            # chunk/group schedule: small first chunk primes the pipeline;
            # stock groups are emitted late so they never block later L0s
            # in the Vector FIFO.
            CHUNKS = [512, 1536, 2048, 2048, 2048]
            GROUP_AT = {3: (0, 16), 4: (16, 16), 5: (32, 32)}  # after chunk i
            # persistent full-width psum tiles; A fills while B drains
            psA = psp.tile([128, 2048], mybir.dt.float32, tag="psA")
            psB = psp.tile([128, 2048], mybir.dt.float32, tag="psB")
            xoff = 0
            for ci, CW in enumerate(CHUNKS):
                xa = xp.tile([128, CW], MM_DT, tag=f"xa{CW}")
                xb = xp.tile([128, CW], MM_DT, tag=f"xb{CW}")
                nc.sync.dma_start(out=xa[:, :], in_=xt.ap()[0, :, xoff : xoff + CW])
                nc.sync.dma_start(out=xb[:, :], in_=xt.ap()[1, :, xoff : xoff + CW])

                uA = up.tile([128, CW], TREE_DT, tag=f"uA{CW}")
                uB = up.tile([128, CW], TREE_DT, tag=f"uB{CW}")
                for half, ps, u in ((0, psA, uA), (1, psB, uB)):
                    for ki in range(2):
                        xsrc = xa if ki == 0 else xb
                        st = wsb[:, 2 * half + ki, :]
                        for f in range(0, CW, MMF):
                            fw = min(MMF, CW - f)
                            nc.tensor.matmul(
                                ps[:, f : f + fw],
                                st,
                                xsrc[:, f : f + fw],
                                start=(ki == 0),
                                stop=(ki == 1),
                            )
                    nc.scalar.activation(
                        out=u[:, :],
                        in_=ps[:, 0:CW],
                        func=mybir.ActivationFunctionType.Sigmoid,
                        bias=bias_shift[:, :],
                        scale=float(SHARPNESS),
                    )

                # L0 custom: v0 = (a0*uA + uB) + ch0*max(a0*uA, uB)
                v0 = vp.tile([128, CW], TREE_DT, tag=f"v0{CW}")
                nc.vector._custom_dve(
                    TREELEAF,
                    out=v0[:, :],
                    in0=uA[:, :],
                    in1=uB[:, :],
                    s0=ppc_sb[:, 0:1],
                    s1=ppc_sb[:, 1:2],
                )
                # rescale onto the stock sigma chain (tensor_scalar, 4x)
                v0f = vp.tile([128, CW], TREE_DT, tag=f"v0f{CW}")
                nc.vector.tensor_scalar(
                    out=v0f[:, :],
                    in0=v0[:, :],
                    scalar1=ppc_sb[:, 2:3],
                    scalar2=None,
                    op0=mybir.AluOpType.mult,
                )
                # transpose [128, CW] -> batch-major [128, CW/128, 128]
                toff = xoff // 128
                nc.sync.dma_start_transpose(
                    v0T[:, toff : toff + CW // 128, :], v0f[:, :]
                )
                xoff += CW
                if ci + 1 in GROUP_AT:
                    t0, tn = GROUP_AT[ci + 1]
                    stock_group(slice(t0, t0 + tn), tn)

            final = constp.tile([128, TILES], mybir.dt.float32)
            nc.scalar.activation(
                out=final[:, :],
                in_=roots[:, :],
                func=mybir.ActivationFunctionType.Sigmoid,
                bias=bias_out[:, :],
                scale=1.0,
            )
            nc.sync.dma_start(out=outp.ap(), in_=final[:, :])

    nc.compile()
    return nc


def make_in_maps(x, W_leaf, weights, biases, w_out):
    import ml_dtypes

    np_mm = ml_dtypes.bfloat16
    cst = prep_consts(weights, biases, w_out)
    br128 = _bitrev(128)
    lA = 2 * br128
    lB = lA + 1

    Wf = W_leaf.astype(np.float32)
    WA = Wf[lA]  # [128 j, 256 k]
    WB = Wf[lB]
    wst = np.empty((128, 4, 128), np.float32)
    wst[:, 0, :] = WA[:, 0:128].T
    wst[:, 1, :] = WA[:, 128:256].T
    wst[:, 2, :] = WB[:, 0:128].T
    wst[:, 3, :] = WB[:, 128:256].T
    wst = np.ascontiguousarray(wst.astype(np_mm))

    ppc = np.zeros((128, 3), np.float32)
    ppc[:, 0] = cst["a0"]
    ppc[:, 1] = cst["ch0"]
    ppc[:, 2] = cst["fix0"]

    cst_row = np.zeros(128, np.float16)
    cst_row[0:126] = cst["chat_cat"].astype(np.float16)
    cst_np = np.ascontiguousarray(np.broadcast_to(cst_row, (128, 128)))

    xT = np.ascontiguousarray(x.T.astype(np_mm))  # [256, B]
    in_maps = []
    for c in range(N_CORES):
        sh = np.ascontiguousarray(xT[:, c * BS : (c + 1) * BS].reshape(2, 128, BS))
        in_maps.append({"xt": sh, "wst": wst, "ppc": ppc, "cst": cst_np})
    return in_maps, cst["a7"], cst["c7"]


def gather_out(results):
    full = np.empty((B, 1), np.float32)
    for c in range(N_CORES):
        r = np.asarray(results[c]["out"])  # [128, TILES]
        full[c * BS : (c + 1) * BS, 0] = r.T.reshape(BS)
    return full


def kernel(x, W_leaf, weights, biases, w_out, b_out, _run_kwargs=None):
    x = np.asarray(x, dtype=np.float32)
    W_leaf = np.asarray(W_leaf, dtype=np.float32)
    weights = np.asarray(weights, dtype=np.float32)
    biases = np.asarray(biases, dtype=np.float32)
    w_out = np.asarray(w_out, dtype=np.float32)
    b_out = np.asarray(b_out, dtype=np.float32)
    in_maps, a7, c7 = make_in_maps(x, W_leaf, weights, biases, w_out)
    nc = build_nc(float(b_out[0]), a7, c7)
    kw = dict(_run_kwargs or {})
    res = run_bass_kernel_spmd(nc, in_maps, core_ids=list(range(N_CORES)), **kw)
    out = gather_out(res.results)
    if _run_kwargs is not None:
        kernel.last_results = res
    return out
